# revision 40
# baseline (speedup 1.0000x reference)
"""Trainium2 Bass kernel for nn_BodyFaceEmotionClassifier.

Pipeline (per reference):
  concat(body, hand_r, hand_l) -> [B,T,67,3]; gate (x,y) by conf>0.1 ->
  pos [B,T,134]; relu(pos@W1+b1); masked max pool over valid t;
  BatchNorm over batch; classifier @Wc+bc -> [64, 7].

Strategy (8 NeuronCores, pure data parallel over batch):
  * Host specializes on the runtime `length` values: batches sorted by
    length, dealt into 8 slots x 8 cores; slot j has one compile-time
    length L_j (group max rounded to 128) so a single SPMD program fits
    every core.  Short batches are padded by repeating their own first
    row (duplicates never change a max-pool).
  * Layout is chosen so the device does NO transposes for the main 128
    features: the host ships them already feature-major ("maint"
    [256, V]: rows 0:64 x0..63, 64:128 y0..63, 128:256 conf c0..63
    twice).  The duplicated conf block makes gating a single dense
    [128, n] fused (conf>thr)*coord DVE op per chunk (DVE partition
    bases of all operands must be equal, so y cannot reuse the x conf
    rows).  The gate COMPARES in full fp32 and writes a float32r tile,
    satisfying the walrus rule that fp32r-matmul inputs be fp32r-
    rounded by their producer.
  * Matmuls run in float32r: 1 PE cycle/row at free-dim 512 (vs 4 for
    plain fp32) with near-fp32 accuracy, since inputs are exact fp32
    gated values rounded once to fp32r.
  * The 6 leftover features (x64..66, y64..66) ship feature-major
    [6, V] already gated on the host (0.1 of one percent of the
    FLOPs); they feed the K=6 accumulating matmul directly -- no PE
    transposes, no Scalar drain, no second gate pass.
  * Per 1024-row chunk: 3 DMAs -> gate -> per 512-sub 2 K-splits x 2
    D-halves matmuls into a 2-bank [128, 1024] PSUM tile per half ->
    ONE free-dim reduce_max per half per chunk on DVE (amortizes the
    PSUM access latency).  bias+relu after pooling (commute with max).
  * AllGather (8KB/rank) of per-core pooled [256, 8]; every core
    redundantly computes BN stats + classifier for all 64 batches; host
    takes core 0's [64, 7] and undoes the sort permutation.
"""

import sys

for _p in ("/opt/trn_rl_repo", "/opt/trn_rl_repo/concourse"):
    if _p not in sys.path:
        sys.path.insert(0, _p)

import ml_dtypes
import numpy as np

NP16 = np.float16

import concourse.bacc as bacc
import concourse.mybir as mybir
import concourse.tile as tile
from concourse import bass_utils

F32 = mybir.dt.float32
F16 = mybir.dt.float16
BF16 = mybir.dt.bfloat16
AX = mybir.AxisListType
OP = mybir.AluOpType
ACT = mybir.ActivationFunctionType

B, T = 64, 4096
K = 67          # keypoints
NF = 134        # 2K gated coord features
NRAW = 201      # 3K raw features
D = 256
C = 7
THR = 0.1
EPS = 1e-5
NCORES = 8
P = 128
KM = 128        # main contraction rows (x0..63, y0..63)
RK = 6          # remainder contraction rows (x64..66, y64..66)
CHUNK = 1024
SUB = 512


def _plan(lengths):
    """Sort batches desc, deal into 8 slots x 8 cores, pad slot length to
    the group max rounded up to a multiple of 128."""
    order = np.argsort(-lengths, kind="stable")
    L = []
    assign = np.empty((NCORES, NCORES), dtype=np.int64)  # [core, slot] -> batch
    for j in range(NCORES):
        grp = order[NCORES * j : NCORES * (j + 1)]
        L.append(int(-(-int(lengths[grp].max()) // P) * P))
        for c in range(NCORES):
            assign[c, j] = grp[c]
    return L, assign


def _chunks(Lj):
    off = 0
    while off < Lj:
        n = min(CHUNK, Lj - off)
        yield off, n
        off += n


def _subs(n):
    off = 0
    while off < n:
        s = min(SUB, n - off)
        yield off, s
        off += s


def _nchunks(L):
    return sum(1 for Lj in L for _ in _chunks(Lj))


def _stream(L):
    """Uniform CHUNK-sized tiles over the whole packed stream, decoupled
    from slot boundaries (so the compute pipeline never hiccups at short
    slot tails).  Yields (roff, n, segs) with segs = [(a, b, j, done)]:
    half-open column ranges [a, b) within the chunk belonging to slot j;
    done marks the segment that finishes slot j."""
    V = sum(L)
    bounds = []
    s = 0
    for Lj in L:
        bounds.append((s, s + Lj))
        s += Lj
    roff = 0
    while roff < V:
        n = min(CHUNK, V - roff)
        segs = []
        for j, (s0, s1) in enumerate(bounds):
            a = max(s0, roff)
            b = min(s1, roff + n)
            if a < b:
                segs.append((a - roff, b - roff, j, b == s1))
        yield roff, n, segs
        roff += n


def _build(L, stop_after="full"):
    """Build + compile the SPMD Bass program for slot lengths L."""
    V = sum(L)
    nseg = sum(len(segs) for _, _, segs in _stream(L))

    nc = bacc.Bacc(
        "TRN2", target_bir_lowering=False, debug=False, num_devices=NCORES
    )

    maint_d = nc.dram_tensor("maint", [P, 2 * V], F16, kind="ExternalInput")
    remg_d = nc.dram_tensor("remg", [RK, V], F16, kind="ExternalInput")
    w1a_d = nc.dram_tensor("w1a", [KM, D], F16, kind="ExternalInput")
    w1b_d = nc.dram_tensor("w1b", [P, D], F16, kind="ExternalInput")
    b1_d = nc.dram_tensor("b1", [D, 1], F32, kind="ExternalInput")
    gamma_d = nc.dram_tensor("gamma", [D, 1], F32, kind="ExternalInput")
    beta_d = nc.dram_tensor("beta", [D, 1], F32, kind="ExternalInput")
    wc_d = nc.dram_tensor("wc", [D, C], F32, kind="ExternalInput")
    bc_d = nc.dram_tensor("bc", [B, C], F32, kind="ExternalInput")
    out_d = nc.dram_tensor("out", [B, C], F32, kind="ExternalOutput")

    with tile.TileContext(nc) as tc:
        with (
            tc.tile_pool(name="consts", bufs=1) as consts,
            tc.tile_pool(name="dram", bufs=1, space="DRAM") as dram,
            tc.tile_pool(name="apool", bufs=7) as apool,
            tc.tile_pool(name="gpool", bufs=4) as gpool,
            tc.tile_pool(name="hpool", bufs=4) as hpool,
            tc.tile_pool(name="psS", bufs=2, space="PSUM") as psS,
        ):
            # consts load on the Activation HWDGE queue, keeping the Sync
            # queue free for the chunk stream
            w1a = consts.tile([KM, D], F16)
            nc.scalar.dma_start(w1a[:], w1a_d[:, :])
            w1b = consts.tile([P, D], F16)
            nc.scalar.dma_start(w1b[:], w1b_d[:, :])
            # rem moving-operand ring: K padded 6 -> 128 with persistent
            # zero rows (a K<128 matmul reconfigures the PE array and
            # serializes the stream at ~2.5x cost; a full-K matmul with
            # zero rows runs at full rate).  DMA fills rows 0:6 per chunk;
            # rows 6:128 stay zero forever.
            NRG = 6
            rgz = [
                consts.tile([P, CHUNK], F16, name=f"rgz{i}")
                for i in range(NRG)
            ]
            for t in rgz:
                nc.gpsimd.memset(t[:], 0.0)
            b1h = consts.tile([P, 2], F32)
            nc.scalar.dma_start(b1h[:, 0:1], b1_d[0:P, :])
            nc.scalar.dma_start(b1h[:, 1:2], b1_d[P:D, :])
            gamh = consts.tile([P, 2], F32)
            nc.scalar.dma_start(gamh[:, 0:1], gamma_d[0:P, :])
            nc.scalar.dma_start(gamh[:, 1:2], gamma_d[P:D, :])
            beth = consts.tile([P, 2], F32)
            nc.scalar.dma_start(beth[:, 0:1], beta_d[0:P, :])
            nc.scalar.dma_start(beth[:, 1:2], beta_d[P:D, :])
            wch = consts.tile([P, 2 * C], F32)
            nc.scalar.dma_start(wch[:, 0:C], wc_d[0:P, :])
            nc.scalar.dma_start(wch[:, C : 2 * C], wc_d[P:D, :])
            bc_sb = consts.tile([B, C], F32)
            nc.scalar.dma_start(bc_sb[:], bc_d[:, :])

            percol = [
                consts.tile([P, nseg], F32, name=f"percol{h}")
                for h in range(2)
            ]
            pooled = [
                consts.tile([P, NCORES], F32, name=f"pooled{h}")
                for h in range(2)
            ]

            ci = 0
            cidx = 0
            slot_c0 = [None] * len(L)
            warmed = False
            for roff, n, segs in _stream(L):
                # ONE interleaved [128, 2n] load per chunk (coords then
                # conf) on the Sync queue; pre-gated rem [6, n] on the
                # GpSimd SWDGE queue into the zero-padded ring
                mt = apool.tile([P, 2 * CHUNK], F16, name="mt", tag="mt")
                rg = rgz[cidx % NRG]
                cidx += 1
                if cidx == 1:
                    # stripe chunk 0 so the pipeline fills sooner
                    half = n  # 2n cols split in two
                    nc.sync.dma_start(
                        mt[:, 0:half], maint_d[:, 0:half]
                    )
                    nc.sync.dma_start(
                        mt[:, half : 2 * n], maint_d[:, half : 2 * n]
                    )
                else:
                    nc.sync.dma_start(
                        mt[:, 0 : 2 * n],
                        maint_d[:, 2 * roff : 2 * roff + 2 * n],
                    )
                nc.sync.dma_start(
                    rg[0:RK, 0:n], remg_d[:, roff : roff + n]
                )
                if not warmed and stop_after in ("collective", "full"):
                    # warm-up AllGather: pays the one-time CC mesh setup
                    # in the shadow of the main loop.  Gated on chunk 0's
                    # load (via the wu_in copy) so the CC rings don't
                    # compete with the pipeline ramp-up.
                    warmed = True
                    wu_in = dram.tile([1, 8], F16)
                    wu_out = dram.tile([NCORES, 1, 8], F16)
                    nc.sync.dma_start(wu_in[:, :], mt[0:1, 0:8])
                    nc.gpsimd.collective_compute(
                        "AllGather",
                        OP.bypass,
                        replica_groups=[list(range(NCORES))],
                        ins=[wu_in[:].opt()],
                        outs=[wu_out[:].opt()],
                    )
                # main gate: one dense fused op [128, n]; conf compares
                # against the fp32 scalar, gated coords written fp16
                pt = gpool.tile([P, CHUNK], F16, name="pt", tag="pt")
                nc.vector.scalar_tensor_tensor(
                    out=pt[:, 0:n],
                    in0=mt[:, n : 2 * n],
                    scalar=THR,
                    in1=mt[:, 0:n],
                    op0=OP.is_gt,
                    op1=OP.mult,
                )
                sf = [
                    psS.tile([P, CHUNK], F32, name=f"sf{h}", tag=f"s{h}")
                    for h in range(2)
                ]
                for so, sn in _subs(n):
                    for h in range(2):
                        nc.tensor.matmul(
                            sf[h][:, so : so + sn],
                            w1a[:, h * P : (h + 1) * P],
                            pt[:, so : so + sn],
                            start=True,
                            stop=False,
                        )
                        nc.tensor.matmul(
                            sf[h][:, so : so + sn],
                            w1b[:, h * P : (h + 1) * P],
                            rg[:, so : so + sn],
                            start=False,
                            stop=True,
                        )
                sfh = [None, None]
                for h in range(2):
                    # drain PSUM to fp16 SBUF on the (otherwise idle)
                    # Scalar engine: frees the PSUM banks early and halves
                    # the DVE read cost; monotone rounding commutes with
                    # the max-pool
                    sfh[h] = hpool.tile(
                        [P, CHUNK], F16, name=f"sfh{h}", tag=f"sfh{h}"
                    )
                    nc.scalar.copy(sfh[h][:, 0:n], sf[h][:, 0:n])
                for a, b, j, done in segs:
                    if slot_c0[j] is None:
                        slot_c0[j] = ci
                    for h in range(2):
                        nc.vector.tensor_reduce(
                            percol[h][:, ci : ci + 1],
                            sfh[h][:, a:b],
                            axis=AX.X,
                            op=OP.max,
                        )
                    ci += 1
                    if done:
                        # slot complete: fold its percol columns
                        for h in range(2):
                            nc.vector.tensor_reduce(
                                pooled[h][:, j : j + 1],
                                percol[h][:, slot_c0[j] : ci],
                                axis=AX.X,
                                op=OP.max,
                            )
            assert ci == nseg
            if stop_after == "mainloop":
                nc.sync.dma_start(out_d[:, :], pooled[0][0:B, 0:C])

            # bias + relu (commute with max-pool)
            prelu = [
                consts.tile([P, NCORES], F32, name=f"prelu{h}")
                for h in range(2)
            ]
            for h in range(2):
                nc.scalar.activation(
                    prelu[h][:],
                    pooled[h][:],
                    ACT.Relu,
                    bias=b1h[:, h : h + 1],
                    scale=1.0,
                )
            if stop_after == "prelu":
                nc.sync.dma_start(out_d[:, :], prelu[0][0:B, 0:C])

            # AllGather pooled [256, 8] across the 8 cores
            if stop_after in ("collective", "full"):
                pool_dt = dram.tile([D, NCORES], F32)
                gath_d = dram.tile([NCORES, D, NCORES], F32)
                for h in range(2):
                    nc.sync.dma_start(
                        pool_dt[h * P : (h + 1) * P, :], prelu[h][:]
                    )
                nc.gpsimd.collective_compute(
                    "AllGather",
                    OP.bypass,
                    replica_groups=[list(range(NCORES))],
                    ins=[pool_dt[:].opt()],
                    outs=[gath_d[:].opt()],
                )
            if stop_after == "collective":
                csb = consts.tile([B, C], F32)
                nc.sync.dma_start(csb[:], gath_d[0, 0:B, 0:C])
                nc.sync.dma_start(out_d[:, :], csb[:])

            # epilogue: BN stats over all 64, normalize, classify (every
            # core redundantly computes the full [64, 7])
            if stop_after == "full":
                gsb = [
                    consts.tile([P, B], F32, name=f"gsb{h}") for h in range(2)
                ]
                for h in range(2):
                    nc.sync.dma_start(
                        gsb[h][:].rearrange("p (r s) -> p r s", r=NCORES),
                        gath_d[:, h * P : (h + 1) * P, :].transpose([1, 0, 2]),
                    )
                epsc = consts.tile([P, 1], F32)
                nc.vector.memset(epsc[:], EPS)
                stats = consts.tile([P, 20], F32)
                sqs = [
                    consts.tile([P, B], F32, name=f"sq{h}") for h in range(2)
                ]
                bnT = [
                    consts.tile([P, B], F32, name=f"bnT{h}") for h in range(2)
                ]
                for h in range(2):
                    sq = sqs[h]
                    o = 10 * h
                    ssum = stats[:, o + 0 : o + 1]
                    mean = stats[:, o + 1 : o + 2]
                    esq = stats[:, o + 2 : o + 3]
                    msq = stats[:, o + 3 : o + 4]
                    var = stats[:, o + 4 : o + 5]
                    sd = stats[:, o + 5 : o + 6]
                    rstd = stats[:, o + 6 : o + 7]
                    scl = stats[:, o + 7 : o + 8]
                    nc.vector.tensor_reduce(
                        ssum, gsb[h][:], axis=AX.X, op=OP.add
                    )
                    nc.vector.tensor_scalar_mul(mean, ssum, 1.0 / B)
                    nc.scalar.activation(sq[:], gsb[h][:], ACT.Square)
                    nc.vector.tensor_reduce(esq, sq[:], axis=AX.X, op=OP.add)
                    nc.vector.tensor_mul(msq, mean, mean)
                    # var = E[x^2] - mean^2 = esq/B - msq
                    nc.vector.scalar_tensor_tensor(
                        out=var,
                        in0=esq,
                        scalar=1.0 / B,
                        in1=msq,
                        op0=OP.mult,
                        op1=OP.subtract,
                    )
                    nc.scalar.activation(sd, var, ACT.Sqrt, bias=epsc[:])
                    nc.vector.reciprocal(rstd, sd)
                    nc.vector.tensor_mul(scl, gamh[:, h : h + 1], rstd)
                    # shift = beta - mean*scl ; bn = gsb*scl + shift
                    ms = sq[:, 0:1]
                    shift = sq[:, 1:2]
                    nc.vector.tensor_mul(ms, mean, scl)
                    nc.vector.tensor_sub(shift, beth[:, h : h + 1], ms)
                    nc.scalar.activation(
                        bnT[h][:], gsb[h][:], ACT.Identity,
                        bias=shift, scale=scl,
                    )
                out_ps = psS.tile([B, C], F32, name="ops", tag="s0")
                nc.tensor.matmul(
                    out_ps[:], bnT[0][:], wch[:, 0:C], start=True, stop=False
                )
                nc.tensor.matmul(
                    out_ps[:], bnT[1][:], wch[:, C : 2 * C],
                    start=False, stop=True,
                )
                osb = consts.tile([B, C], F32)
                nc.vector.tensor_add(osb[:], out_ps[:], bc_sb[:])
                nc.sync.dma_start(out_d[:, :], osb[:])

    nc.compile()
    return nc, V


_CACHE = {}


def _get_program(L):
    key = tuple(L)
    if key not in _CACHE:
        _CACHE[key] = _build(list(L))
    return _CACHE[key]


def _nudge_conf16(cf32):
    """fp16-round conf so the device's (conf > 0.1) predicate matches the
    fp32 reference exactly, whether the scalar compares as fp32(0.1) or
    fp16(0.1): above-threshold values are forced strictly above fp32(0.1)
    and the rest to at most fp16(0.1) (the smaller of the two)."""
    want = cf32 > np.float32(THR)
    ch = cf32.astype(NP16)
    chf = ch.astype(np.float32)
    lo = NP16(THR)                      # 0.0999755859375 <= both thresholds
    hi = np.nextafter(lo, NP16(np.inf))  # 0.10003662109375 > fp32(0.1)
    ch = np.where(want & ~(chf > np.float32(THR)), hi, ch)
    ch = np.where(~want & (chf > lo.astype(np.float32)), lo, ch)
    return ch


def _pack_inputs(body, hand_right, hand_left, lengths, L, assign, V):
    """Per-core inputs (all fp16): maint [128, 2V] with per-chunk
    interleaving -- for each 1024-col chunk at row-offset r, columns
    2r:2r+n hold the coords (rows x0..63,y0..63) and columns 2r+n:2r+2n
    hold the conf rows (c0..63 twice, nudged to preserve the >0.1
    predicate) -- so one DMA brings a whole chunk.  remg [6, V] is
    feature-major pre-gated (x64..66, y64..66 times (conf>0.1)).
    Padding rows repeat the batch's first row."""
    maint_all, remg_all = [], []
    chunk_bounds = [(roff, n) for roff, n, _ in _stream(L)]
    assert sum(n for _, n in chunk_bounds) == V
    for c in range(NCORES):
        buf = np.empty((V, NRAW), dtype=np.float32)
        off = 0
        for j, Lj in enumerate(L):
            b = int(assign[c, j])
            lb = int(lengths[b])
            row = np.concatenate(
                (body[b, :lb], hand_right[b, :lb], hand_left[b, :lb]), axis=1
            )
            buf[off : off + lb] = row
            if Lj > lb:
                buf[off + lb : off + Lj] = row[0]
            off += Lj
        assert off == V
        coords = np.empty((P, V), dtype=NP16)
        coords[0:64] = buf[:, 0 : 3 * 64 : 3].T.astype(NP16)   # x0..63
        coords[64:128] = buf[:, 1 : 3 * 64 : 3].T.astype(NP16)  # y0..63
        confs = np.empty((P, V), dtype=NP16)
        confs[0:64] = _nudge_conf16(buf[:, 2 : 3 * 64 : 3].T)     # c0..63
        confs[64:128] = confs[0:64]
        maint = np.empty((P, 2 * V), dtype=NP16)
        for r, n in chunk_bounds:
            maint[:, 2 * r : 2 * r + n] = coords[:, r : r + n]
            maint[:, 2 * r + n : 2 * r + 2 * n] = confs[:, r : r + n]
        g = (buf[:, 194:201:3] > THR).astype(np.float32)  # c64..66 gate
        remg = np.empty((RK, V), dtype=NP16)
        remg[0:3] = (buf[:, 192:201:3] * g).T.astype(NP16)  # x64..66
        remg[3:6] = (buf[:, 193:201:3] * g).T.astype(NP16)  # y64..66
        maint_all.append(np.ascontiguousarray(maint))
        remg_all.append(np.ascontiguousarray(remg))
    return maint_all, remg_all


def _make_base(W1, b1, gamma, beta, Wc, bc):
    W1 = np.asarray(W1, dtype=np.float32)
    # w1a row order matches maint rows: x0..63 -> W1[2k], y0..63 -> W1[2k+1]
    w1a = np.concatenate((W1[0 : 2 * 64 : 2], W1[1 : 2 * 64 : 2]), axis=0)
    # w1b row order matches remg rows: x64..66 -> W1[2k], y64..66 -> W1[2k+1];
    # zero-padded to K=128 (rows 6:128) to keep the PE pipeline full-rate
    w1b = np.zeros((P, D), dtype=np.float32)
    w1b[0:3] = W1[2 * 64 :: 2]
    w1b[3:6] = W1[2 * 64 + 1 :: 2]
    return {
        "w1a": np.ascontiguousarray(w1a.astype(NP16)),
        "w1b": np.ascontiguousarray(w1b.astype(NP16)),
        "b1": np.asarray(b1, np.float32).reshape(D, 1).copy(),
        "gamma": np.asarray(gamma, np.float32).reshape(D, 1).copy(),
        "beta": np.asarray(beta, np.float32).reshape(D, 1).copy(),
        "wc": np.ascontiguousarray(np.asarray(Wc, np.float32)),
        "bc": np.broadcast_to(
            np.asarray(bc, np.float32).reshape(1, C), (B, C)
        ).copy(),
    }


def kernel(body, hand_right, hand_left, length, W1, b1, gamma, beta, Wc, bc):
    lengths = np.asarray(length).astype(np.int64)
    body = np.asarray(body, dtype=np.float32)
    hand_right = np.asarray(hand_right, dtype=np.float32)
    hand_left = np.asarray(hand_left, dtype=np.float32)

    L, assign = _plan(lengths)
    nc, V = _get_program(L)
    maint_all, remg_all = _pack_inputs(
        body, hand_right, hand_left, lengths, L, assign, V
    )
    base = _make_base(W1, b1, gamma, beta, Wc, bc)
    in_maps = [
        dict(base, maint=maint_all[c], remg=remg_all[c])
        for c in range(NCORES)
    ]

    res = bass_utils.run_bass_kernel_spmd(
        nc, in_maps, core_ids=list(range(NCORES))
    )
    kernel.last_results = res
    out_sorted = res.results[0]["out"]  # row r*8+s = batch assign[r, s]

    out = np.empty((B, C), dtype=np.float32)
    for r in range(NCORES):
        for s in range(NCORES):
            out[int(assign[r, s])] = out_sorted[r * NCORES + s]
    return out
